# revision 1
# baseline (speedup 1.0000x reference)
"""Trainium2 Bass kernel for DigitCapsules dynamic routing.

Problem: u [256, 2048, 8] f32, W [1, 2048, 10, 16, 8] f32
  u_hat = einsum('pcoi,bpi->bpco', W[0], u)
  3 routing iterations (softmax over c, weighted sum over p, squash,
  agreement update) -> v [256, 10, 16] f32.

Strategy (8 cores, data-parallel over batch, 32 batch elems per core):
  - Partition layout: slabs of 16 p-values; SBUF partition index =
    (p_local * 8 + b_member); the PE contraction runs over
    K = (p_local 16, i 8) = 128 via a block-diagonal stationary u_bd
    (host-built; the zeros cost nothing at matmul time).
  - u_hat materialized per 8-batch group in SBUF bf16, layout
    [part=(p,b), free=(slab, o, c)] -- c innermost so the big DVE ops hit
    the 2x bf16 mode; PSUM evacuation does the (c,o)->(o,c) transpose.
  - 4 groups are software-pipelined: group k+1's u_hat matmuls run while
    group k routes, keeping the PE warm (HAM) and engines overlapped.
  - Iteration 1 needs no u_hat read: s1 = 0.1 * sum_p u_hat from a
    dense-u stationary matmul accumulated over all slabs.
  - Iterations 2,3: G via DVE/GpSimd mul + DVE add-tree over o; softmax
    over c via ACT exp + DVE reduce; weighted s-sum via PE matmuls with
    a block-diagonal ones stationary (row b of the accumulator).
  - Host arrays are k-major so every DMA is contiguous; DMAs spread
    across the SP / Activation / GpSimd queues.
"""

import numpy as np
import ml_dtypes

bf16 = ml_dtypes.bfloat16

# Problem constants (fixed by the problem spec; do not read spec.json here)
B, P, C, O, IN = 256, 2048, 10, 16, 8
NCORES = 8
B_LOC = B // NCORES          # 32 batch elems per core
BT = 8                       # batch elems per group (one octet)
NGROUP = B_LOC // BT         # 4 groups per core
PSLAB = 16                   # p-values per slab
NSLAB = P // PSLAB           # 128 slabs
CO = C * O                   # 160
ROUTING_ITERS = 3
EPS = 1e-9

CHB = 8     # slabs per u_bd DMA chunk
EV = 3      # slabs per PSUM evacuation batch (1 bank per tile)
SMM = 2     # slabs per s-step matmul (N = SMM*CO = 320 <= 512)


def _host_prep(u_core, W0, nslab=NSLAB, ngroup=NGROUP):
    """Build host-side reordered (k-major, contiguous-DMA) arrays."""
    # w_k[p*8+i, s, c*16+o] = W0[16s+p, c, o, i]
    w = W0.reshape(nslab, PSLAB, C, O, IN)
    w_k = np.ascontiguousarray(
        w.transpose(1, 4, 0, 2, 3).reshape(PSLAB * IN, nslab, CO)
    ).astype(bf16)

    # x[g, b, s, p, i] = u_core[g*8 + b, 16s+p, i]
    x = u_core.reshape(ngroup, BT, nslab, PSLAB, IN)

    # ubd_k[g, p*8+i, s, p'*8+b] = x[g,b,s,p,i] * (p == p')
    xt = x.transpose(0, 3, 4, 2, 1)  # [g, p, i, s, b]
    ubd_k = np.zeros((ngroup, PSLAB, IN, nslab, PSLAB, BT), dtype=bf16)
    for p in range(PSLAB):
        ubd_k[:, p, :, :, p, :] = xt[:, p]
    ubd_k = ubd_k.reshape(ngroup, PSLAB * IN, nslab, PSLAB * BT)

    # ut_k[p*8+i, s, g*8+b] = x[g,b,s,p,i] -- one dense stationary for the
    # s1 matmul covering ALL groups (M = ngroup*BT)
    ut_k = np.ascontiguousarray(
        x.transpose(3, 4, 2, 0, 1).reshape(PSLAB * IN, nslab, ngroup * BT)
    ).astype(bf16)

    # ones_bd[p*8+b, b'] = (b == b') -- stationary for the s-reduction
    ones_bd = np.zeros((PSLAB * BT, BT), dtype=bf16)
    for p in range(PSLAB):
        for b in range(BT):
            ones_bd[p * BT + b, b] = 1.0
    return {
        "w_k": w_k,
        "ubd_k": ubd_k,
        "ut_k": ut_k,
        "ones_bd": ones_bd,
    }


def build(nc, tc, ctx, nslab=NSLAB, ngroup=NGROUP):
    """Emit the kernel IR. Parameterized slab/group counts for small tests."""
    import concourse.bass as bass
    from concourse import mybir

    f32 = mybir.dt.float32
    i32 = mybir.dt.int32
    bf = mybir.dt.bfloat16
    Alu = mybir.AluOpType
    Act = mybir.ActivationFunctionType
    Ax = mybir.AxisListType

    b_loc = ngroup * BT
    chb = min(CHB, nslab)
    ev = min(EV, nslab)
    smm = min(SMM, nslab)

    # ---- DRAM parameters ----
    w_dram = nc.dram_tensor(
        "w_k", [PSLAB * IN, nslab, CO], bf, kind="ExternalInput").ap()
    ubd_dram = nc.dram_tensor(
        "ubd_k", [ngroup, PSLAB * IN, nslab, PSLAB * BT], bf,
        kind="ExternalInput").ap()
    ut_dram = nc.dram_tensor(
        "ut_k", [PSLAB * IN, nslab, ngroup * BT], bf,
        kind="ExternalInput").ap()
    ones_dram = nc.dram_tensor(
        "ones_bd", [PSLAB * BT, BT], bf, kind="ExternalInput").ap()
    vout_dram = nc.dram_tensor(
        "v_out", [b_loc, CO], f32, kind="ExternalOutput").ap()
    vscr_dram = nc.dram_tensor("v_scratch", [ngroup, BT, CO], bf).ap()

    # ---- pools ----
    consts = ctx.enter_context(tc.tile_pool(name="consts", bufs=1))
    ubdpool = ctx.enter_context(tc.tile_pool(name="ubdpool", bufs=2))
    utpool = ctx.enter_context(tc.tile_pool(name="utpool", bufs=1))
    uhatpool = ctx.enter_context(tc.tile_pool(name="uhat", bufs=2))
    psum = ctx.enter_context(tc.tile_pool(name="psum", bufs=4, space="PSUM"))
    psum_acc = ctx.enter_context(
        tc.tile_pool(name="psum_acc", bufs=2, space="PSUM"))
    small = ctx.enter_context(tc.tile_pool(name="small", bufs=2))
    state = ctx.enter_context(tc.tile_pool(name="state", bufs=2))
    tmp = ctx.enter_context(tc.tile_pool(name="tmp", bufs=2))

    ones_sb = consts.tile([PSLAB * BT, BT], bf)
    nc.sync.dma_start(out=ones_sb[:], in_=ones_dram)
    magic = consts.tile([128, 1], i32)
    nc.gpsimd.memset(magic[:], 0x5F3759DF)

    v1_keep = [consts.tile([BT, O * C], bf, tag=f"v1k{g}",
                           name=f"v1k{g}") for g in range(ngroup)]

    # resident W: whole tensor, two contiguous halves on the ACT queue
    wall = consts.tile([PSLAB * IN, nslab, CO], bf)
    h = max(1, nslab // 4)
    for j in range(0, nslab, h):
        nc.scalar.dma_start(
            out=wall[:, j:j + h, :], in_=w_dram[:, j:j + h, :])

    def bcast_ap(ap, insert_pos, size):
        """Insert a stride-0 dim of `size` at free-dim position insert_pos."""
        new = list(ap.ap)
        new.insert(insert_pos, [0, size])
        return bass.AP(tensor=ap.tensor, offset=ap.offset, ap=new)

    def squash_and_store(s_sb, n, g, it):
        """s_sb: [n, CO] f32 (layout (c,o)), n rows starting at batch g*BT.
        v = squash(s). it < last: writes v (o,c-major bf16) to vscr_dram
        rows; else DMAs v f32 to v_out rows."""
        s3 = s_sb[:].rearrange("n (c o) -> n c o", c=C)
        sq = small.tile([n, CO], f32, tag="sqsq", bufs=1)
        nc.vector.tensor_mul(sq[:].rearrange("n (c o) -> n c o", c=C), s3, s3)
        nrm = small.tile([n, C], f32, tag="nrm")
        nc.vector.tensor_reduce(
            out=nrm[:], in_=sq[:].rearrange("n (c o) -> n c o", c=C),
            axis=Ax.X, op=Alu.add)
        d1 = small.tile([n, C], f32, tag="d1")
        nc.vector.tensor_scalar_add(d1[:], nrm[:], 1.0)
        r1 = small.tile([n, C], f32, tag="r1")
        nc.vector.reciprocal(r1[:], d1[:])
        se = small.tile([n, C], f32, tag="se")
        nc.vector.tensor_scalar_add(se[:], nrm[:], EPS)
        # rsqrt(se) = int bit-trick seed + 2 Newton steps (all DVE; keeps
        # the scalar engine on a single activation table)
        sh = small.tile([n, C], i32, tag="sh")
        nc.vector.tensor_scalar(
            out=sh[:], in0=se[:].bitcast(i32), scalar1=1, scalar2=None,
            op0=Alu.logical_shift_right)
        y0 = small.tile([n, C], i32, tag="y0")
        nc.vector.tensor_tensor(
            out=y0[:], in0=bcast_ap(magic[0:n, :], 1, C), in1=sh[:],
            op=Alu.subtract)
        y = y0[:].bitcast(f32)
        for tgA in ("na", "nb"):
            aa = small.tile([n, C], f32, tag=tgA)
            nc.vector.tensor_tensor(out=aa[:], in0=y, in1=y, op=Alu.mult)
            nc.vector.tensor_tensor(out=aa[:], in0=aa[:], in1=se[:],
                                    op=Alu.mult)
            nc.vector.tensor_scalar(
                out=aa[:], in0=aa[:], scalar1=-0.5, scalar2=1.5,
                op0=Alu.mult, op1=Alu.add)
            yn = small.tile([n, C], f32, tag=tgA + "y")
            nc.vector.tensor_tensor(out=yn[:], in0=y, in1=aa[:], op=Alu.mult)
            y = yn[:]
        r2 = small.tile([n, C], f32, tag="r2")
        nc.vector.tensor_copy(out=r2[:], in_=y)
        f1 = small.tile([n, C], f32, tag="f1")
        nc.vector.tensor_mul(f1[:], nrm[:], r1[:])
        fac = small.tile([n, C], f32, tag="fac")
        nc.vector.tensor_mul(fac[:], f1[:], r2[:])
        v_sb = small.tile([n, CO], f32, tag="v_sb")
        nc.vector.tensor_tensor(
            out=v_sb[:].rearrange("n (c o) -> n c o", c=C),
            in0=s3, in1=bcast_ap(fac[:], 2, O), op=Alu.mult)
        if it == ROUTING_ITERS - 1:
            nc.sync.dma_start(
                out=vout_dram[g * BT:g * BT + n, :], in_=v_sb[:])
            return
        # v_bf stored (o,c)-major so the V_rep broadcast DMA is 3-dim
        v_bf = small.tile([n, O * C], bf, tag="v_bf")
        nc.vector.tensor_copy(
            v_bf[:].rearrange("n (o c) -> n c o", o=O),
            v_sb[:].rearrange("n (c o) -> n c o", c=C))
        if it > 0:
            # routing squash: store accumulated V = v1 + v2 so the next
            # iteration's logits come out of one linear G pass
            nc.vector.tensor_tensor(
                out=v_bf[:], in0=v_bf[:],
                in1=v1_keep[g][0:n, :], op=Alu.add)
        nc.sync.dma_start(
            out=vscr_dram.rearrange("g n x -> (g n) x")[
                g * BT:g * BT + n, :], in_=v_bf[:])
        return v_bf

    def load_vrep(V_rep, g):
        src = bass.AP(
            tensor=vscr_dram.tensor,
            offset=vscr_dram.offset + g * BT * CO,
            ap=[[0, PSLAB], [CO, BT], [1, O * C]])
        nc.sync.dma_start(out=V_rep[:], in_=src)

    def load_vrep_sbuf(V_rep, v_bf):
        # replicate v (one octet) across the 16 p-positions, SBUF->SBUF
        src = bass.AP(
            tensor=v_bf[:].tensor, offset=v_bf[:].offset,
            ap=[[0, PSLAB]] + list(v_bf[:].ap))
        nc.sync.dma_start(out=V_rep[:], in_=src)

    # ---------- s1 sweep: one accumulation for ALL groups (M=32) ----------
    ut_res = utpool.tile([PSLAB * IN, nslab, ngroup * BT], bf, tag="utres",
                         name="ut_res")
    nc.sync.dma_start(out=ut_res[:], in_=ut_dram)
    s1_ps = psum_acc.tile([ngroup * BT, CO], f32, tag="s1ps", name="s1ps")
    for s in range(nslab):
        nc.tensor.matmul(
            out=s1_ps[:], lhsT=ut_res[:, s, :], rhs=wall[:, s, :],
            start=(s == 0), stop=(s == nslab - 1))
    s1_sb = consts.tile([ngroup * BT, CO], f32)
    nc.scalar.mul(s1_sb[:], s1_ps[:], 1.0 / C)
    squash_and_store(s1_sb, ngroup * BT, 0, 0)
    for g in range(ngroup):
        nc.sync.dma_start(
            out=v1_keep[g][:],
            in_=vscr_dram.rearrange("g n x -> (g n) x")[
                g * BT:(g + 1) * BT, :])

    def phase_a(g):
        # ---------- Phase A: u_hat materialization ----------
        uhat = uhatpool.tile([128, nslab, O, C], bf, tag="uhat", name="uhat")
        ubs = {}

        def get_ub(ci):
            if ci not in ubs:
                ub = ubdpool.tile([PSLAB * IN, chb, PSLAB * BT], bf,
                                  tag="ubd", name="ubd")
                eng = nc.gpsimd if ci % 2 else nc.sync
                eng.dma_start(
                    out=ub[:], in_=ubd_dram[g, :, ci * chb:(ci + 1) * chb, :])
                ubs[ci] = ub
            return ubs[ci]

        s0 = 0
        while s0 < nslab:
            nb = min(ev, nslab - s0)
            ps = psum.tile([128, ev, CO], f32, tag="ups", name="ups")
            get_ub(s0 // chb)
            get_ub((s0 + nb - 1) // chb)
            for q in range(nb):
                sl = s0 + q
                ub = ubs[sl // chb]
                nc.tensor.matmul(
                    out=ps[:, q, :], lhsT=ub[:, sl % chb, :],
                    rhs=wall[:, sl, :], start=True, stop=True)
            src = ps[:, 0:nb, :].rearrange("p s (c o) -> p s o c", c=C)
            dst = uhat[:, s0:s0 + nb, :, :]
            nc.scalar.copy(dst, src)
            s0 += nb
        return uhat

    def route_it(g, uhat, V_rep, it):
        if V_rep is None:
            # iteration 1: v1 precomputed, just replicate
            V_rep = state.tile([128, O, C], bf, tag="vrep", name="vrep",
                               bufs=3)
            load_vrep(V_rep, g)

        # b-state scratch (bf16: G magnitudes are <<1)
        bst = state.tile([128, nslab, C], bf, tag="bst", name="bst")

        nchunk = max(1, nslab // 32)
        chs = nslab // nchunk  # slabs per compute chunk

        if True:
            s_ps = psum_acc.tile([BT, smm * CO], f32, tag="sps", name="sps")
            # ---- G-step: bst (+)= sum_o uhat * V_rep ----
            for ch in range(nchunk):
                sl = slice(ch * chs, (ch + 1) * chs)
                t2 = tmp.tile([128, chs, O, C], bf, tag="t2", bufs=3)
                nc.vector.tensor_tensor(
                    out=t2[:], in0=uhat[:, sl, :, :],
                    in1=bcast_ap(V_rep[:], 1, chs), op=Alu.mult)
                r1 = tmp.tile([128, chs, O // 2, C], bf, tag="r1t")
                nc.vector.tensor_tensor(
                    out=r1[:], in0=t2[:, :, 0:O // 2, :],
                    in1=t2[:, :, O // 2:O, :], op=Alu.add)
                r2 = tmp.tile([128, chs, O // 4, C], bf, tag="r2t")
                nc.vector.tensor_tensor(
                    out=r2[:], in0=r1[:, :, 0:O // 4, :],
                    in1=r1[:, :, O // 4:O // 2, :], op=Alu.add)
                r3 = tmp.tile([128, chs, 2, C], bf, tag="r3t")
                nc.vector.tensor_tensor(
                    out=r3[:], in0=r2[:, :, 0:2, :],
                    in1=r2[:, :, 2:4, :], op=Alu.add)
                nc.vector.tensor_tensor(
                    out=bst[:, sl, :], in0=r3[:, :, 0, :],
                    in1=r3[:, :, 1, :], op=Alu.add)
            # ---- softmax over c, chunked to pipeline with G and s ----
            expt = tmp.tile([128, nslab, C], bf, tag="expt", bufs=2)
            Z = tmp.tile([128, nslab], f32, tag="Z")
            rz = tmp.tile([128, nslab], f32, tag="rz")
            cw = tmp.tile([128, nslab, C], bf, tag="cw", bufs=2)
            for hf in range(nchunk):
                hs = slice(hf * chs, (hf + 1) * chs)
                nc.scalar.activation(expt[:, hs, :], bst[:, hs, :], Act.Exp)
                nc.vector.tensor_reduce(
                    out=Z[:, hs], in_=expt[:, hs, :], axis=Ax.X, op=Alu.add)
                nc.vector.reciprocal(rz[:, hs], Z[:, hs])
                nc.vector.tensor_tensor(
                    out=cw[:, hs, :], in0=expt[:, hs, :],
                    in1=bcast_ap(rz[:, hs], 2, C), op=Alu.mult)
            # ---- s-step: premul + PE block-diag ones reduction ----
            for ch in range(nchunk):
                sl = slice(ch * chs, (ch + 1) * chs)
                t1 = tmp.tile([128, chs, O, C], bf, tag="t2", bufs=3)
                nc.vector.tensor_tensor(
                    out=t1[:], in0=uhat[:, sl, :, :],
                    in1=bcast_ap(cw[:, sl, :], 2, O), op=Alu.mult)
                for k in range(chs // smm):
                    s_idx = ch * chs + k * smm
                    nc.tensor.matmul(
                        out=s_ps[:], lhsT=ones_sb[:],
                        rhs=t1[:, k * smm:(k + 1) * smm, :, :],
                        start=(s_idx == 0),
                        stop=(s_idx == nslab - smm))
            # collect s: sum the smm slab-positions; each is (o,c) ordered
            s_sb = small.tile([BT, CO], f32, tag="s_sb")
            if smm == 2:
                s_rw = small.tile([BT, 2 * CO], f32, tag="s_rw")
                nc.scalar.copy(s_rw[:], s_ps[:])
                nc.vector.tensor_tensor(
                    out=s_sb[:].rearrange("n (c o) -> n c o", c=C),
                    in0=s_rw[:, 0:CO].rearrange("n (o c) -> n c o", o=O),
                    in1=s_rw[:, CO:2 * CO].rearrange("n (o c) -> n c o", o=O),
                    op=Alu.add)
            else:
                nc.scalar.copy(
                    s_sb[:].rearrange("n (c o) -> n c o", c=C),
                    s_ps[:, 0:CO].rearrange("n (o c) -> n c o", o=O))
            v_bf = squash_and_store(s_sb, BT, g, it)
            if it < ROUTING_ITERS - 1:
                V_rep_next = state.tile([128, O, C], bf, tag="vrep",
                                        name="vrepn", bufs=3)
                load_vrep(V_rep_next, g)
                return V_rep_next
            return None

    # Paired emission: both it1's of a group pair precede their it2's so
    # two independent routing units stay in flight at the pipeline drain
    # (only 2 uhat tiles live at a time -> uhat bufs=2 still works).
    for gp in range(0, ngroup, 2):
        ua = phase_a(gp)
        ub = phase_a(gp + 1)
        va = route_it(gp, ua, None, 1)
        vb = route_it(gp + 1, ub, None, 1)
        route_it(gp, ua, va, 2)
        route_it(gp + 1, ub, vb, 2)


def make_inputs_per_core(u, W):
    """Full inputs -> list of 8 in_maps."""
    W0 = np.asarray(W, dtype=np.float32)[0]
    u = np.asarray(u, dtype=np.float32)
    in_maps = []
    for c in range(NCORES):
        u_core = u[c * B_LOC:(c + 1) * B_LOC]
        in_maps.append(_host_prep(u_core, W0))
    return in_maps


def numpy_model(u_core, W0):
    """f32 numpy model of the routing (for small-scale checks)."""
    u_hat = np.einsum('pcoi,bpi->bpco', W0, u_core)
    b = np.zeros(u_hat.shape[:3], dtype=np.float32)
    v = None
    for _ in range(ROUTING_ITERS):
        e = np.exp(b - b.max(axis=2, keepdims=True))
        c = e / e.sum(axis=2, keepdims=True)
        s = np.einsum('bpc,bpco->bco', c, u_hat)
        sq = (s * s).sum(-1, keepdims=True)
        v = (sq / (1 + sq)) * s / np.sqrt(sq + EPS)
        b = b + np.einsum('bpco,bco->bpc', u_hat, v)
    return v


_COMPILED = {}


def _get_compiled():
    if "nc" in _COMPILED:
        return _COMPILED["nc"]
    from contextlib import ExitStack
    import concourse.tile as tile
    from concourse import bacc

    nc = bacc.Bacc("TRN2", target_bir_lowering=False, debug=False,
                   num_devices=NCORES)
    with tile.TileContext(nc) as tc:
        with ExitStack() as ctx:
            build(nc, tc, ctx)
    nc.compile()
    _COMPILED["nc"] = nc
    return nc


def kernel(u, W):
    """Full-input entry point: u [256,2048,8] f32, W [1,2048,10,16,8] f32
    -> v [256, 10, 16] f32."""
    from concourse.bass_utils import run_bass_kernel_spmd

    nc = _get_compiled()
    in_maps = make_inputs_per_core(u, W)
    res = run_bass_kernel_spmd(nc, in_maps, core_ids=list(range(NCORES)))
    outs = [res.results[c]["v_out"] for c in range(NCORES)]
    v = np.concatenate(outs, axis=0).reshape(B, C, O).astype(np.float32)
    return v



# revision 20
# speedup vs baseline: 1.0449x; 1.0449x over previous
"""Trainium2 Bass kernel for DigitCapsules dynamic routing (v2).

Problem: u [256, 2048, 8] f32, W [1, 2048, 10, 16, 8] f32
  u_hat = einsum('pcoi,bpi->bpco', W[0], u)
  3 routing iterations (softmax over c, weighted sum over p, squash,
  agreement update) -> v [256, 10, 16] f32.

Strategy (8 cores, data-parallel over batch, 32 batch elems per core):
  - Partition layout: slabs of 16 p-values; SBUF partition index =
    (p_local * 8 + b_member); the PE contraction runs over
    K = (p_local 16, i 8) = 128 via a block-diagonal stationary u_bd.
  - v2: u_bd is built ON-CHIP: two persistent buffers are memset to zero
    once, then each group DMAs only the 16 diagonal strips (dense u) --
    ~1MB instead of 16.8MB of host-built zeros.
  - v2: W columns are (o,c)-ordered host-side, so PSUM arrives as
    [slab, o, c] and evacuation is a straight (non-transposing) copy.
  - u_hat in SBUF bf16, layout [part=(p,b), free=(slab, o, c)] -- c
    innermost so the big DVE ops hit the 2x bf16 mode.
  - Iteration 1 needs no u_hat read: s1 = 0.1 * sum_p u_hat from a
    dense-u stationary matmul accumulated over all slabs.
  - Iterations 2,3: G via DVE mul + add-tree over o; softmax over c via
    ACT exp + DVE reduce; weighted s-sum via PE matmuls with a
    block-diagonal ones stationary.
  - v2: v never roundtrips through DRAM: V_rep broadcasts and the v1
    accumulator are SBUF->SBUF DMAs; squash is pair-batched ([16,160]).
"""

import numpy as np
import ml_dtypes

bf16 = ml_dtypes.bfloat16

# Problem constants (fixed by the problem spec; do not read spec.json here)
B, P, C, O, IN = 256, 2048, 10, 16, 8
NCORES = 8
B_LOC = B // NCORES          # 32 batch elems per core
BT = 8                       # batch elems per group (one octet)
NGROUP = B_LOC // BT         # 4 groups per core
PSLAB = 16                   # p-values per slab
NSLAB = P // PSLAB           # 128 slabs
CO = C * O                   # 160
ROUTING_ITERS = 3
EPS = 1e-9

EV = 3      # slabs per PSUM evacuation batch (1 bank per tile)
SMM = 2     # slabs per s-step matmul (N = SMM*CO = 320 <= 512)


def _host_prep(u_core, W0, nslab=NSLAB, ngroup=NGROUP):
    """Build host-side reordered (k-major, contiguous-DMA) arrays."""
    # w_k[p*8+i, s, o*10+c] = W0[16s+p, c, o, i]  ((o,c)-ordered columns)
    w = W0.reshape(nslab, PSLAB, C, O, IN)
    w_k = np.ascontiguousarray(
        w.transpose(1, 4, 0, 3, 2).reshape(PSLAB * IN, nslab, CO)
    ).astype(bf16)

    # x[g, b, s, p, i] = u_core[g*8 + b, 16s+p, i]
    x = u_core.reshape(ngroup, BT, nslab, PSLAB, IN)

    # ud_k[g, p, i, s, b] = x[g,b,s,p,i]  -- dense diagonal strips for the
    # on-chip block-diagonal build
    ud_k = np.ascontiguousarray(x.transpose(0, 3, 4, 2, 1)).astype(bf16)

    # ut_k[p*8+i, s, g*8+b] = x[g,b,s,p,i] -- one dense stationary for the
    # s1 matmul covering ALL groups (M = ngroup*BT)
    ut_k = np.ascontiguousarray(
        x.transpose(3, 4, 2, 0, 1).reshape(PSLAB * IN, nslab, ngroup * BT)
    ).astype(bf16)

    # ones2[j, p*8+b, j'*8+b'] = (b == b') & (j == j') -- stationaries for
    # the s-reduction; group j of a pair fills output rows j*8..j*8+8 while
    # contributing zeros to the other half (PSUM-accumulated jointly).
    ones2 = np.zeros((2, PSLAB * BT, 2 * BT), dtype=bf16)
    for j in range(2):
        for p in range(PSLAB):
            for b in range(BT):
                ones2[j, p * BT + b, j * BT + b] = 1.0
    return {
        "w_k": w_k,
        "ud_k": ud_k,
        "ut_k": ut_k,
        "ones2": ones2,
    }


def build(nc, tc, ctx, nslab=NSLAB, ngroup=NGROUP):
    """Emit the kernel IR. Parameterized slab/group counts for small tests."""
    import concourse.bass as bass
    from concourse import mybir

    f32 = mybir.dt.float32
    i32 = mybir.dt.int32
    bf = mybir.dt.bfloat16
    Alu = mybir.AluOpType
    Act = mybir.ActivationFunctionType
    Ax = mybir.AxisListType

    b_loc = ngroup * BT
    ev = min(EV, nslab)
    smm = min(SMM, nslab)
    npair = max(1, ngroup // 2)

    # ---- DRAM parameters ----
    w_dram = nc.dram_tensor(
        "w_k", [PSLAB * IN, nslab, CO], bf, kind="ExternalInput").ap()
    ud_dram = nc.dram_tensor(
        "ud_k", [ngroup, PSLAB, IN, nslab, BT], bf,
        kind="ExternalInput").ap()
    ut_dram = nc.dram_tensor(
        "ut_k", [PSLAB * IN, nslab, ngroup * BT], bf,
        kind="ExternalInput").ap()
    ones_dram = nc.dram_tensor(
        "ones2", [2, PSLAB * BT, 2 * BT], bf, kind="ExternalInput").ap()
    vout_dram = nc.dram_tensor(
        "v_out", [b_loc, CO], f32, kind="ExternalOutput").ap()

    # ---- pools ----
    consts = ctx.enter_context(tc.tile_pool(name="consts", bufs=1))
    uhatpool = ctx.enter_context(tc.tile_pool(name="uhat", bufs=2))
    psum = ctx.enter_context(tc.tile_pool(name="psum", bufs=4, space="PSUM"))
    psum_acc = ctx.enter_context(
        tc.tile_pool(name="psum_acc", bufs=2, space="PSUM"))
    small = ctx.enter_context(tc.tile_pool(name="small", bufs=2))
    state = ctx.enter_context(tc.tile_pool(name="state", bufs=2))
    tmp = ctx.enter_context(tc.tile_pool(name="tmp", bufs=2))

    ones_sb = consts.tile([PSLAB * BT, 2, 2 * BT], bf)
    for j in range(2):
        nc.sync.dma_start(out=ones_sb[:, j, :], in_=ones_dram[j])
    magic = consts.tile([128, 1], i32)
    nc.gpsimd.memset(magic[:], 0x5F3759DF)

    # persistent on-chip block-diagonal u buffers (zeros persist; each
    # chunk's DMA overwrites only the 16 diagonal strips)
    chb = min(32, nslab)
    ubd_bufs = [consts.tile([PSLAB * IN, chb, PSLAB * BT], bf,
                            name=f"ubd{j}") for j in range(2)]
    nc.vector.memset(ubd_bufs[0][:], 0.0)
    nc.gpsimd.memset(ubd_bufs[1][:], 0.0)

    # v1 accumulator per group-pair [16, O*C] (o,c)-major bf16
    v1k = [consts.tile([2 * BT, O * C], bf, name=f"v1k{j}")
           for j in range(npair)]

    # resident W: whole tensor, contiguous eighths across queues
    wall = consts.tile([PSLAB * IN, nslab, CO], bf)
    h = max(1, nslab // 8)
    engs = [nc.scalar, nc.sync, nc.gpsimd]
    for jj, j in enumerate(range(0, nslab, h)):
        engs[jj % 3].dma_start(
            out=wall[:, j:j + h, :], in_=w_dram[:, j:j + h, :])

    def bcast_ap(ap, insert_pos, size):
        """Insert a stride-0 dim of `size` at free-dim position insert_pos."""
        new = list(ap.ap)
        new.insert(insert_pos, [0, size])
        return bass.AP(tensor=ap.tensor, offset=ap.offset, ap=new)

    def part_bcast_ap(ap, reps):
        """Prepend a stride-0 PARTITION dim of `reps` (replicates the
        ap's partition rows across reps blocks)."""
        new = [[0, reps]] + list(ap.ap)
        return bass.AP(tensor=ap.tensor, offset=ap.offset, ap=new)

    def squash(s_sb, n, it, vk=None):
        """s_sb: [n, CO] f32, (o,c)-major. Returns v tiles.
        it < last: returns v_bf [n, O*C] bf16 (o,c) with accumulated V
        (v1 + v) when vk given; else final: returns v_sb f32 (o,c)."""
        s3 = s_sb[:].rearrange("n (o c) -> n c o", o=O)
        sq = small.tile([n, CO], f32, tag="sqsq", bufs=1)
        nc.vector.tensor_mul(sq[:].rearrange("n (o c) -> n c o", o=O), s3, s3)
        nrm = small.tile([n, C], f32, tag="nrm")
        nc.vector.tensor_reduce(
            out=nrm[:], in_=sq[:].rearrange("n (o c) -> n c o", o=O),
            axis=Ax.X, op=Alu.add)
        d1 = small.tile([n, C], f32, tag="d1")
        nc.vector.tensor_scalar_add(d1[:], nrm[:], 1.0)
        r1 = small.tile([n, C], f32, tag="r1")
        nc.vector.reciprocal(r1[:], d1[:])
        se = small.tile([n, C], f32, tag="se")
        nc.vector.tensor_scalar_add(se[:], nrm[:], EPS)
        # rsqrt(se) = int bit-trick seed + 2 Newton steps (all DVE)
        sh = small.tile([n, C], i32, tag="sh")
        nc.vector.tensor_scalar(
            out=sh[:], in0=se[:].bitcast(i32), scalar1=1, scalar2=None,
            op0=Alu.logical_shift_right)
        y0 = small.tile([n, C], i32, tag="y0")
        nc.vector.tensor_tensor(
            out=y0[:], in0=bcast_ap(magic[0:n, :], 1, C), in1=sh[:],
            op=Alu.subtract)
        y = y0[:].bitcast(f32)
        for tgA in ("na", "nb"):
            aa = small.tile([n, C], f32, tag=tgA)
            nc.vector.tensor_tensor(out=aa[:], in0=y, in1=y, op=Alu.mult)
            nc.vector.tensor_tensor(out=aa[:], in0=aa[:], in1=se[:],
                                    op=Alu.mult)
            nc.vector.tensor_scalar(
                out=aa[:], in0=aa[:], scalar1=-0.5, scalar2=1.5,
                op0=Alu.mult, op1=Alu.add)
            yn = small.tile([n, C], f32, tag=tgA + "y")
            nc.vector.tensor_tensor(out=yn[:], in0=y, in1=aa[:], op=Alu.mult)
            y = yn[:]
        f1 = small.tile([n, C], f32, tag="f1")
        nc.vector.tensor_mul(f1[:], nrm[:], r1[:])
        fac = small.tile([n, C], f32, tag="fac")
        nc.vector.tensor_mul(fac[:], f1[:], y)
        v_sb = small.tile([n, CO], f32, tag="v_sb")
        nc.vector.tensor_tensor(
            out=v_sb[:].rearrange("n (o c) -> n c o", o=O),
            in0=s3, in1=bcast_ap(fac[:], 2, O), op=Alu.mult)
        if it == ROUTING_ITERS - 1:
            return v_sb
        v_bf = small.tile([n, O * C], bf, tag="v_bf")
        if vk is not None:
            # accumulated V = v1 + v2 so the next iteration's logits come
            # out of one linear G pass
            nc.vector.tensor_tensor(
                out=v_bf[:],
                in0=v_sb[:],  # f32 + bf16 -> bf16 cast on write
                in1=vk[0:n, :], op=Alu.add)
        else:
            nc.vector.tensor_copy(out=v_bf[:], in_=v_sb[:])
        return v_bf

    def load_vrep(V_rep, v_bf, r0):
        """Replicate v rows [r0:r0+8] across the 16 p-positions (16 small
        SBUF->SBUF block-copy DMAs spread over the queues)."""
        for p in range(PSLAB):
            eng = (nc.sync, nc.gpsimd, nc.scalar)[p % 3]
            eng.dma_start(
                out=V_rep[p * BT:(p + 1) * BT, :, :]
                    .rearrange("n o c -> n (o c)"),
                in_=v_bf[r0:r0 + BT, :])

    # ---------- s1 sweep: one accumulation for ALL groups (M=32) ----------
    ut_res = consts.tile([PSLAB * IN, nslab, ngroup * BT], bf, name="ut_res")
    nc.sync.dma_start(out=ut_res[:, 0:nslab // 2, :],
                      in_=ut_dram[:, 0:nslab // 2, :])
    nc.gpsimd.dma_start(out=ut_res[:, nslab // 2:, :],
                        in_=ut_dram[:, nslab // 2:, :])
    s1_ps = psum_acc.tile([ngroup * BT, CO], f32, tag="s1ps", name="s1ps")
    for s in range(nslab):
        nc.tensor.matmul(
            out=s1_ps[:], lhsT=ut_res[:, s, :], rhs=wall[:, s, :],
            start=(s == 0), stop=(s == nslab - 1))
    s1_sb = consts.tile([ngroup * BT, CO], f32)
    nc.scalar.mul(s1_sb[:], s1_ps[:], 1.0 / C)
    v_bf1 = squash(s1_sb, ngroup * BT, 0)
    # v1 accumulator pair tiles (partition rows 0..15) <- rows of v_bf1
    for j in range(npair):
        r = min(2 * BT, b_loc - j * 2 * BT)
        nc.scalar.dma_start(out=v1k[j][0:r, :],
                            in_=v_bf1[j * 2 * BT:j * 2 * BT + r, :])

    # diagonal-strip DMAs into a persistent (pre-zeroed) ubd chunk buffer
    ubd_ctr = [0]

    def load_ubd_chunk(g, c0):
        ub = ubd_bufs[ubd_ctr[0] % 2]
        ubd_ctr[0] += 1
        for p in range(PSLAB):
            eng = (nc.sync, nc.gpsimd, nc.scalar)[p % 3]
            eng.dma_start(
                out=ub[p * IN:(p + 1) * IN, :, p * BT:(p + 1) * BT],
                in_=ud_dram[g, p, :, c0:c0 + chb, :])
        return ub

    def phase_a(g):
        # ---------- Phase A: u_hat materialization ----------
        uhat = uhatpool.tile([128, nslab, O, C], bf, tag="uhat", name="uhat")
        for c0 in range(0, nslab, chb):
            ub = load_ubd_chunk(g, c0)
            s0 = c0
            while s0 < c0 + chb:
                nb = min(ev, c0 + chb - s0)
                ps = psum.tile([128, ev, CO], f32, tag="ups", name="ups")
                for q in range(nb):
                    sl = s0 + q
                    nc.tensor.matmul(
                        out=ps[:, q, :], lhsT=ub[:, sl - c0, :],
                        rhs=wall[:, sl, :], start=True, stop=True)
                # (o,c)-ordered W columns -> straight PSUM->SBUF copy
                nc.scalar.copy(
                    uhat[:, s0:s0 + nb, :, :]
                        .rearrange("p s o c -> p (s o c)"),
                    ps[:, 0:nb, :].rearrange("p s x -> p (s x)"))
                s0 += nb
        return uhat

    def route_core(g, uhat, V_rep, s_ps, j, jlast):
        """One group's G/softmax/premul/s-matmul for one iteration.
        The s-matmuls accumulate into the pair-shared s_ps [16, smm*CO]
        via the ones2[j] stationary (rows j*8..j*8+8)."""
        # b-state scratch (bf16: G magnitudes are <<1)
        bst = state.tile([128, nslab, C], bf, tag="bst", name="bst")
        nchunk = max(1, nslab // 32)
        chs = nslab // nchunk  # slabs per compute chunk
        # ---- G-step: bst = sum_o uhat * V_rep ----
        for ch in range(nchunk):
            sl = slice(ch * chs, (ch + 1) * chs)
            t2 = tmp.tile([128, chs, O, C], bf, tag="t2", bufs=2)
            nc.vector.tensor_tensor(
                out=t2[:], in0=uhat[:, sl, :, :],
                in1=bcast_ap(V_rep[:], 1, chs), op=Alu.mult)
            r1 = tmp.tile([128, chs, O // 2, C], bf, tag="r1t")
            nc.vector.tensor_tensor(
                out=r1[:], in0=t2[:, :, 0:O // 2, :],
                in1=t2[:, :, O // 2:O, :], op=Alu.add)
            r2 = tmp.tile([128, chs, O // 4, C], bf, tag="r2t")
            nc.vector.tensor_tensor(
                out=r2[:], in0=r1[:, :, 0:O // 4, :],
                in1=r1[:, :, O // 4:O // 2, :], op=Alu.add)
            r3 = tmp.tile([128, chs, 2, C], bf, tag="r3t")
            nc.vector.tensor_tensor(
                out=r3[:], in0=r2[:, :, 0:2, :],
                in1=r2[:, :, 2:4, :], op=Alu.add)
            nc.vector.tensor_tensor(
                out=bst[:, sl, :], in0=r3[:, :, 0, :],
                in1=r3[:, :, 1, :], op=Alu.add)
        # ---- softmax over c, chunked to pipeline with G and s ----
        expt = tmp.tile([128, nslab, C], bf, tag="expt", bufs=2)
        Z = tmp.tile([128, nslab], f32, tag="Z", bufs=1)
        rz = tmp.tile([128, nslab], f32, tag="rz", bufs=1)
        cw = tmp.tile([128, nslab, C], bf, tag="cw", bufs=2)
        for hf in range(nchunk):
            hs = slice(hf * chs, (hf + 1) * chs)
            nc.scalar.activation(expt[:, hs, :], bst[:, hs, :], Act.Exp)
            nc.vector.tensor_reduce(
                out=Z[:, hs], in_=expt[:, hs, :], axis=Ax.X, op=Alu.add)
            nc.vector.reciprocal(rz[:, hs], Z[:, hs])
            nc.vector.tensor_tensor(
                out=cw[:, hs, :], in0=expt[:, hs, :],
                in1=bcast_ap(rz[:, hs], 2, C), op=Alu.mult)
        # ---- s-step: premul + PE block-diag ones reduction ----
        for ch in range(nchunk):
            sl = slice(ch * chs, (ch + 1) * chs)
            t1 = tmp.tile([128, chs, O, C], bf, tag="t2", bufs=2)
            nc.vector.tensor_tensor(
                out=t1[:], in0=uhat[:, sl, :, :],
                in1=bcast_ap(cw[:, sl, :], 2, O), op=Alu.mult)
            for k in range(chs // smm):
                s_idx = ch * chs + k * smm
                nc.tensor.matmul(
                    out=s_ps[:], lhsT=ones_sb[:, j, :],
                    rhs=t1[:, k * smm:(k + 1) * smm, :, :],
                    start=(j == 0 and s_idx == 0),
                    stop=(j == jlast and s_idx == nslab - smm))

    def collect_s(s_ps, s_pair, nrow):
        # collect s: sum the smm slab-positions ((o,c)-ordered already)
        if smm == 2:
            s_rw = small.tile([nrow, 2 * CO], f32, tag="s_rw")
            nc.scalar.copy(s_rw[:], s_ps[0:nrow, :])
            nc.vector.tensor_tensor(
                out=s_pair[:],
                in0=s_rw[:, 0:CO], in1=s_rw[:, CO:2 * CO], op=Alu.add)
        else:
            nc.scalar.copy(s_pair[:], s_ps[0:nrow, 0:CO])

    # Paired emission: both groups of a pair route together; squash is
    # batched over the pair ([16,160] per iteration).
    for j in range(npair):
        ga, gb = 2 * j, 2 * j + 1
        two = (gb < ngroup)
        nrow = 2 * BT if two else BT
        ua = phase_a(ga)
        ub = phase_a(gb) if two else None
        # iteration-1 V_rep = v1 rows (from the s1 squash)
        Va = state.tile([128, O, C], bf, tag="vrep", name="vrep", bufs=4)
        load_vrep(Va, v_bf1, ga * BT)
        if two:
            Vb = state.tile([128, O, C], bf, tag="vrep", name="vrepb",
                            bufs=4)
            load_vrep(Vb, v_bf1, gb * BT)
        for it in range(1, ROUTING_ITERS):
            s_ps = psum_acc.tile([2 * BT, smm * CO], f32, tag="sps",
                                 name="sps")
            route_core(ga, ua, Va, s_ps, 0, 1 if two else 0)
            if two:
                route_core(gb, ub, Vb, s_ps, 1, 1)
            s_pair = small.tile([nrow, CO], f32, tag="s_pair", bufs=2)
            collect_s(s_ps, s_pair, nrow)
            last = (it == ROUTING_ITERS - 1)
            if last:
                # v_out rows are (o,c)-major; host transposes after gather
                v_sb = squash(s_pair, nrow, it)
                nc.sync.dma_start(
                    out=vout_dram[ga * BT:ga * BT + nrow, :], in_=v_sb[:])
            else:
                v_bf = squash(s_pair, nrow, it, vk=v1k[j])
                Va = state.tile([128, O, C], bf, tag="vrep", name="vrep2",
                                bufs=4)
                load_vrep(Va, v_bf, 0)
                if two:
                    Vb = state.tile([128, O, C], bf, tag="vrep",
                                    name="vrep2b", bufs=4)
                    load_vrep(Vb, v_bf, BT)


def make_inputs_per_core(u, W):
    """Full inputs -> list of 8 in_maps."""
    W0 = np.asarray(W, dtype=np.float32)[0]
    u = np.asarray(u, dtype=np.float32)
    in_maps = []
    for c in range(NCORES):
        u_core = u[c * B_LOC:(c + 1) * B_LOC]
        in_maps.append(_host_prep(u_core, W0))
    return in_maps


def numpy_model(u_core, W0):
    """f32 numpy model of the routing (for small-scale checks)."""
    u_hat = np.einsum('pcoi,bpi->bpco', W0, u_core)
    b = np.zeros(u_hat.shape[:3], dtype=np.float32)
    v = None
    for _ in range(ROUTING_ITERS):
        e = np.exp(b - b.max(axis=2, keepdims=True))
        c = e / e.sum(axis=2, keepdims=True)
        s = np.einsum('bpc,bpco->bco', c, u_hat)
        sq = (s * s).sum(-1, keepdims=True)
        v = (sq / (1 + sq)) * s / np.sqrt(sq + EPS)
        b = b + np.einsum('bpco,bco->bpc', u_hat, v)
    return v


_COMPILED = {}


def _get_compiled():
    if "nc" in _COMPILED:
        return _COMPILED["nc"]
    from contextlib import ExitStack
    import concourse.tile as tile
    from concourse import bacc

    nc = bacc.Bacc("TRN2", target_bir_lowering=False, debug=False,
                   num_devices=NCORES)
    with tile.TileContext(nc) as tc:
        with ExitStack() as ctx:
            build(nc, tc, ctx)
    nc.compile()
    _COMPILED["nc"] = nc
    return nc


def kernel(u, W):
    """Full-input entry point: u [256,2048,8] f32, W [1,2048,10,16,8] f32
    -> v [256, 10, 16] f32."""
    from concourse.bass_utils import run_bass_kernel_spmd

    nc = _get_compiled()
    in_maps = make_inputs_per_core(u, W)
    res = run_bass_kernel_spmd(nc, in_maps, core_ids=list(range(NCORES)))
    outs = [res.results[c]["v_out"] for c in range(NCORES)]
    # v_out rows are (o,c)-major -> [B, O, C] -> transpose to [B, C, O]
    v = np.concatenate(outs, axis=0).reshape(B, O, C)
    return np.ascontiguousarray(v.transpose(0, 2, 1)).astype(np.float32)
